# revision 1
# baseline (speedup 1.0000x reference)
"""Contrastive loss kernel for Trainium2 (8 NeuronCores).

loss = mean((sim.sum(-1) - diag) / T) with sim = n @ n.T, n = x/||x||
     = (||sum_i n_i||^2 - sum_i ||n_i||^2) / (N*T)
     = (||s||^2 - N) / (N*T)          with s = sum_i x_i / ||x_i||

Each core takes a [2048, 512] row shard (16 row-tiles of [128, 512]).
Row sum-of-squares alternates between VectorE (bn_stats -> D*(var+mean^2),
even tiles) and ScalarE (Square activation with accum_out, odd tiles) to
balance the engines; rnorm = reciprocal(sqrt(ss)) with the sqrt batched
per group. The partial s_local = sum_i rnorm_i * x_i is 16 PE matmuls
(lhsT = rnorm [128,1], rhs = x tile [128,512], float32r for full-rate PE)
accumulated in one PSUM bank, copied to SBUF, and DMA'd out per core as
a [1, 512] vector. The host sums the 8 partials and applies the scalar
epilogue (all-reduce of a [D] vector + scalar math).
"""

import numpy as np

import concourse.bass as bass
import concourse.bacc as bacc
import concourse.tile as tile
from concourse import mybir
from concourse.bass_utils import run_bass_kernel_spmd

N = 16384
D = 512
NCORES = 8
ROWS = N // NCORES   # 2048 rows per core
P = 128              # SBUF partitions
NTILES = ROWS // P   # 16 row-tiles per core
GROUPS = (4, 4, 2, 1, 1, 1, 1, 1, 1)   # rsqrt batch sizes (sum = NTILES)
TEMPERATURE = 0.5

F32 = mybir.dt.float32
F32R = mybir.dt.float32r
SQUARE = mybir.ActivationFunctionType.Square

_NC = None


def _build_nc() -> bass.Bass:
    nc = bacc.Bacc(None)
    x_in = nc.declare_dram_parameter("x", [ROWS, D], F32R, isOutput=False)
    s_out = nc.declare_dram_parameter("s", [1, D], F32, isOutput=True)
    x_t = x_in.rearrange("(t p) d -> p t d", p=P)

    with tile.TileContext(nc) as tc:
        with (
            tc.tile_pool(name="xs", bufs=NTILES) as xs_pool,
            tc.tile_pool(name="sq", bufs=2) as sq_pool,
            tc.tile_pool(name="bn", bufs=4) as bn_pool,
            tc.tile_pool(name="st", bufs=16) as st_pool,
            tc.tile_pool(name="acc", bufs=1, space="PSUM") as psum_pool,
            tc.tile_pool(name="one", bufs=1) as one_pool,
        ):
            acc = psum_pool.tile([1, D], F32)

            xt = [None] * NTILES   # float32r views (PE operands)
            xf = [None] * NTILES   # float32 views of the same bytes (stats)
            for i in range(NTILES):
                x2 = xs_pool.tile([P, D], F32R)
                nc.sync.dma_start(out=x2, in_=x_t[:, i, :])
                xt[i] = x2[:, :]
                xf[i] = x2[:, :].bitcast(F32)

            def emit_stats(t, ss_col):
                if t % 2 == 1:
                    # ScalarE: ss = sum_d x^2 via Square + accumulate
                    sq = sq_pool.tile([P, D], F32)
                    nc.scalar.activation(
                        out=sq, in_=xf[t], func=SQUARE, accum_out=ss_col
                    )
                else:
                    # VectorE: ss = D*(var + mean^2) == sum_d x^2
                    bn6 = bn_pool.tile([P, 6], F32, tag="bn6")
                    nc.vector.bn_stats(out=bn6, in_=xf[t])
                    mv = bn_pool.tile([P, 2], F32, tag="mv")
                    nc.vector.bn_aggr(out=mv, in_=bn6)
                    m2 = bn_pool.tile([P, 1], F32, tag="m2")
                    nc.vector.tensor_mul(m2, mv[:, 0:1], mv[:, 0:1])
                    nc.vector.tensor_scalar(
                        out=ss_col,
                        in0=m2,
                        scalar1=mv[:, 1:2],
                        scalar2=float(D),
                        op0=mybir.AluOpType.add,
                        op1=mybir.AluOpType.mult,
                    )

            rn = [None] * NTILES
            base = 0
            for gsz in GROUPS:
                tiles = range(base, base + gsz)
                base += gsz
                ss = st_pool.tile([P, gsz], F32, tag="ss")
                for j, t in enumerate(tiles):
                    emit_stats(t, ss[:, j : j + 1])
                nc.scalar.sqrt(out=ss, in_=ss)
                r = st_pool.tile([P, gsz], F32R, tag="rn")
                with nc.allow_low_precision(reason="fp32r rounding for PE operands"):
                    nc.vector.reciprocal(out=r, in_=ss)
                for j, t in enumerate(tiles):
                    rn[t] = r[:, j : j + 1]

            for i in range(NTILES):
                nc.tensor.matmul(
                    acc,
                    lhsT=rn[i],
                    rhs=xt[i],
                    start=(i == 0),
                    stop=(i == NTILES - 1),
                )

            res = one_pool.tile([1, D], F32)
            nc.scalar.copy(out=res, in_=acc)
            nc.sync.dma_start(out=s_out[:, :], in_=res)

    nc.finalize()
    return nc


def _run(x: np.ndarray, trace: bool = False):
    global _NC
    if _NC is None:
        _NC = _build_nc()
    x = np.ascontiguousarray(np.asarray(x, dtype=np.float32)).reshape(NCORES, ROWS, D)
    in_maps = [{"x": x[c]} for c in range(NCORES)]
    out = run_bass_kernel_spmd(_NC, in_maps, core_ids=list(range(NCORES)), trace=trace)
    s = np.zeros(D, dtype=np.float64)
    for r in out.results:
        s += r["s"].reshape(D).astype(np.float64)
    loss = (float(s @ s) - float(N)) / (N * TEMPERATURE)
    return np.asarray(loss, dtype=np.float32), out


def kernel(x: np.ndarray) -> np.ndarray:
    loss, _ = _run(x)
    return loss



# revision 5
# speedup vs baseline: 1.2182x; 1.2182x over previous
"""Contrastive loss kernel v4 for Trainium2 (8 NeuronCores).

loss = (||s||^2 - N) / (N*T)  with  s = sum_i x_i / ||x_i||.

Per core [2048, 512] = 16 tiles of [128, 512].

DMA channels (engine-parallel in the cost model):
  Pool (SWDGE): 6 cast-DMAs fp32->fp16 (500ns, priced on output bytes)
  SP   (HWDGE): 6 fp32 loads (790ns)
  ACT  (HWDGE): 4 fp32 loads (790ns)
Sum-of-squares, split by expected arrival time:
  DVE scalar_tensor_tensor (594ns) for the earliest arrivals,
  Pool tensor_tensor squares -> fp16 (427ns) + DVE tensor_scalar accum
  in the 4x perf mode (194ns) for the bulk,
  ACT square+accum (799ns) for two late arrivals.
One explicit LoadActFuncSet (sqrt_and_others: serves Sqrt AND Square)
right after ACT's DMAs, so no implicit table loads block anything.
rnorm = sqrt(1/ss): reciprocal (DVE) -> Sqrt (ACT); mixed batches emit
a second fp16-output Sqrt for the fp16-class matmul operands.
PE: junk 1x1 matmuls reading early ss columns pace the p-state ramp;
all 16 real matmuls then run at full clock (213ns).
Out: Pool copies PSUM->SBUF (427ns), SP DMAs to DRAM.
"""

import numpy as np

import concourse.bass as bass
import concourse.bacc as bacc
import concourse.tile as tile
from concourse import mybir
from concourse.bass_utils import run_bass_kernel_spmd

N = 16384
D = 512
NCORES = 8
ROWS = N // NCORES
P = 128
NTILES = ROWS // P
TEMPERATURE = 0.5

F32 = mybir.dt.float32
F32R = mybir.dt.float32r
F16 = mybir.dt.float16
SQRT = mybir.ActivationFunctionType.Sqrt
SQUARE = mybir.ActivationFunctionType.Square
MULT = mybir.AluOpType.mult
ADD = mybir.AluOpType.add

POOL_TILES = [0, 1, 2, 3, 4, 5]      # fp16 cast path (arr ~2483+500i)
SP_TILES = [6, 7, 8, 9, 10, 11]      # fp32 via SP   (arr ~2707+790i)
ACT_TILES = [12, 13, 14, 15]         # fp32 via ACT  (arr ~3990+790i)

ACT_SQ = [12, 11]                    # ACT square+accum tiles
POOL_SQ = [7, 13, 2, 3, 8, 14, 4, 9, 10, 15]   # Pool-squared, by arrival
# DVE op order (expected data-ready order); ss col = position.
DVE_OPS = [
    ("ttr", 0), ("ttr", 6), ("acc", 7), ("acc", 13),
    ("ttr", 1), ("acc", 2), ("acc", 3), ("ttr", 5),
    ("acc", 8), ("acc", 14), ("acc", 4), ("acc", 9),
    ("acc", 10), ("acc", 15),
]
# ss cols 14, 15 are written by the ACT squares (t12, t11)
BATCHES = [(0, 4), (4, 4), (14, 1), (8, 4), (12, 2), (15, 1)]
N_JUNK_MM = 4
ACT_TABLE_ID = 3                     # 'sqrt_and_others' in act_info.json

_NC = None


def _build_nc() -> bass.Bass:
    nc = bacc.Bacc(None)
    x_in = nc.declare_dram_parameter("x", [ROWS, D], F32R, isOutput=False)
    s_out = nc.declare_dram_parameter("s", [1, D], F32, isOutput=True)
    x_t = x_in.rearrange("(t p) d -> p t d", p=P)

    with tile.TileContext(nc) as tc:
        with (
            tc.tile_pool(name="xh", bufs=len(POOL_TILES)) as xh_pool,
            tc.tile_pool(name="xf", bufs=len(SP_TILES) + len(ACT_TILES)) as xf_pool,
            tc.tile_pool(name="sq", bufs=len(POOL_SQ)) as sq_pool,
            tc.tile_pool(name="scr", bufs=4) as scr_pool,
            tc.tile_pool(name="st", bufs=24) as st_pool,
            tc.tile_pool(name="acc", bufs=2, space="PSUM") as psum_pool,
            tc.tile_pool(name="res", bufs=1) as res_pool,
        ):
            acc = psum_pool.tile([1, D], F32)
            wacc = psum_pool.tile([1, 1], F32)

            xh = [None] * NTILES
            xfr = [None] * NTILES
            xff = [None] * NTILES

            for t in POOL_TILES:
                h = xh_pool.tile([P, D], F16)
                nc.gpsimd.dma_start(out=h, in_=x_t[:, t, :])
                xh[t] = h[:, :]
            for t in SP_TILES:
                f = xf_pool.tile([P, D], F32R)
                nc.sync.dma_start(out=f, in_=x_t[:, t, :])
                xfr[t] = f[:, :]
                xff[t] = f[:, :].bitcast(F32)
            for t in ACT_TILES:
                f = xf_pool.tile([P, D], F32R)
                nc.scalar.dma_start(out=f, in_=x_t[:, t, :])
                xfr[t] = f[:, :]
                xff[t] = f[:, :].bitcast(F32)
            # one explicit act-table load: sqrt_and_others covers Sqrt+Square
            nc.scalar.add_instruction(
                mybir.InstLoadActFuncSet(
                    name=f"I-{nc.next_id()}",
                    act_func_set_id=ACT_TABLE_ID,
                    ins=[],
                    outs=[],
                )
            )

            ss = st_pool.tile([P, NTILES], F32, tag="ss")
            col = {t: j for j, (_, t) in enumerate(DVE_OPS)}
            col[ACT_SQ[0]] = NTILES - 2
            col[ACT_SQ[1]] = NTILES - 1

            sqt = {}
            for t in POOL_SQ:
                s16 = sq_pool.tile([P, D], F16)
                src = xh[t] if xh[t] is not None else xff[t]
                nc.gpsimd.tensor_tensor(out=s16, in0=src, in1=src, op=MULT)
                sqt[t] = s16[:, :]

            scr_h = scr_pool.tile([P, D], F16, tag="scr16")
            scr_f = scr_pool.tile([P, D], F32, tag="scr32")
            sqa = scr_pool.tile([P, D], F32, tag="sqact")
            inv_col = {c: t for t, c in col.items()}
            rn = [None] * NTILES

            def emit_dve(kind, t):
                c = col[t]
                if kind == "acc":
                    nc.vector.tensor_scalar(
                        out=scr_h, in0=sqt[t], scalar1=0.0, scalar2=None,
                        op0=ADD, op1=ADD, accum_out=ss[:, c : c + 1],
                    )
                elif xh[t] is not None:
                    nc.vector.scalar_tensor_tensor(
                        out=scr_h, in0=xh[t], scalar=1.0, in1=xh[t],
                        op0=MULT, op1=MULT, accum_out=ss[:, c : c + 1],
                    )
                else:
                    nc.vector.scalar_tensor_tensor(
                        out=scr_f, in0=xff[t], scalar=1.0, in1=xff[t],
                        op0=MULT, op1=MULT, accum_out=ss[:, c : c + 1],
                    )

            def emit_actsq(t):
                nc.scalar.activation(
                    out=sqa, in_=xff[t], func=SQUARE,
                    accum_out=ss[:, col[t] : col[t] + 1],
                )

            def emit_batch(c0, gsz):
                tiles = [inv_col[c] for c in range(c0, c0 + gsz)]
                # rn = 1/sqrt(ss): Sqrt on ACT, then reciprocal on DVE with a
                # true F32R output (the PE verifier requires f32r operands to
                # be produced rounded, not bitcast).
                sq = st_pool.tile([P, gsz], F32, tag="ssq")
                nc.scalar.activation(out=sq, in_=ss[:, c0 : c0 + gsz], func=SQRT)
                r32 = st_pool.tile([P, gsz], F32R, tag="rn32")
                with nc.allow_low_precision(reason="fp32r rn for PE operands"):
                    nc.vector.reciprocal(out=r32, in_=sq)
                r16 = None
                if any(xh[t] is not None for t in tiles):
                    # PE rejects mixed 32/16-bit matmul operands: fp16-class
                    # tiles need an fp16 lhsT (cheap DVE cast copy).
                    r16 = st_pool.tile([P, gsz], F16, tag="rn16")
                    with nc.allow_low_precision(reason="fp16 rn for fp16 matmuls"):
                        nc.vector.tensor_scalar_add(
                            out=r16, in0=r32.bitcast(F32), scalar1=0.0
                        )
                for j, t in enumerate(tiles):
                    if xh[t] is not None:
                        rn[t] = r16[:, j : j + 1]
                    else:
                        rn[t] = r32[:, j : j + 1]

            # Interleaved emission: per-engine program order approximates the
            # desired execution order (the scheduler mostly preserves it).
            emit_dve(*DVE_OPS[0])
            emit_dve(*DVE_OPS[1])
            emit_dve(*DVE_OPS[2])
            emit_dve(*DVE_OPS[3])
            emit_batch(0, 4)                      # b1: earliest rn -> PE start
            emit_dve(*DVE_OPS[4])
            emit_dve(*DVE_OPS[5])
            emit_dve(*DVE_OPS[6])
            emit_dve(*DVE_OPS[7])
            emit_batch(4, 4)                      # b2
            emit_dve(*DVE_OPS[8])
            emit_dve(*DVE_OPS[9])
            emit_dve(*DVE_OPS[10])
            emit_dve(*DVE_OPS[11])
            emit_batch(8, 4)                      # b3
            emit_dve(*DVE_OPS[12])
            emit_dve(*DVE_OPS[13])
            emit_batch(12, 2)                     # b4
            emit_actsq(ACT_SQ[0])                 # t12 (col 14)
            emit_actsq(ACT_SQ[1])                 # t11 (col 15)
            emit_batch(14, 1)                     # t12's rn
            emit_batch(15, 1)                     # t11's rn

            # PE pacing junk matmuls on early ss columns
            for k in range(N_JUNK_MM):
                nc.tensor.matmul(
                    wacc, lhsT=ss[:, k : k + 1], rhs=ss[:, k : k + 1],
                    start=True, stop=True,
                )

            mm_order = [t for c0, gsz in BATCHES for t in
                        (inv_col[c] for c in range(c0, c0 + gsz))]
            for k, t in enumerate(mm_order):
                rhs = xh[t] if xh[t] is not None else xfr[t]
                nc.tensor.matmul(
                    acc, lhsT=rn[t], rhs=rhs,
                    start=(k == 0), stop=(k == NTILES - 1),
                )

            res = res_pool.tile([1, D], F32)
            nc.scalar.copy(out=res, in_=acc)
            nc.sync.dma_start(out=s_out[:, :], in_=res)

    nc.finalize()
    return nc


def _run(x: np.ndarray, trace: bool = False):
    global _NC
    if _NC is None:
        _NC = _build_nc()
    x = np.ascontiguousarray(np.asarray(x, dtype=np.float32)).reshape(NCORES, ROWS, D)
    in_maps = [{"x": x[c]} for c in range(NCORES)]
    out = run_bass_kernel_spmd(_NC, in_maps, core_ids=list(range(NCORES)), trace=trace)
    s = np.zeros(D, dtype=np.float64)
    for r in out.results:
        s += r["s"].reshape(D).astype(np.float64)
    loss = (float(s @ s) - float(N)) / (N * TEMPERATURE)
    return np.asarray(loss, dtype=np.float32), out


def kernel(x: np.ndarray) -> np.ndarray:
    loss, _ = _run(x)
    return loss
